# revision 4
# baseline (speedup 1.0000x reference)
"""Trainium2 Bass kernel for conv-stack + NetVLAD pooling + linear head.

Pure data parallel: 32 images sharded 4-per-core across 8 NeuronCores.

Per-core pipeline (per image):
  conv1 (3->4, 3x3 SAME, 512x512) as banded bf16 matmuls over (row, ci)
  contraction; relu+w-pool-sum fused into one ACT + one DVE op per tile
  -> Y1 [128=(h%32)*4+c, 16 blk, 256 w] (bf16, sum-pooled).
  conv2 (4->16) as banded bf16 matmuls with pool1's h-direction folded into
  the band; tiles paired for 512-wide moving operands; relu + w-pool-sum
  -> V [128=(r%8)*16+c, 32 t, 128 w] (bf16).
  NetVLAD: logitsT = V.T @ A_blk per tile on the PE (pool2-h folded into
  A_blk, bias added on DVE), softmax over k=4 in the free dim; xf^T from a
  bf16 PE transpose of V + pair-sum on DVE; per-tile Gram via ONE bf16
  matmul with a block-padded lhsT so the q-diagonal blocks land on
  partitions {0,32,64,96}; diagonals summed on DVE; tiny normalize +
  linear finale.
"""
import sys

sys.path.insert(0, "/opt/trn_rl_repo")

import numpy as np
import ml_dtypes
import concourse.bacc as bacc
import concourse.tile as tile
from concourse import mybir
from concourse.bass_utils import run_bass_kernel_spmd

F32 = mybir.dt.float32
BF16 = mybir.dt.bfloat16
AX = mybir.AxisListType
ALU = mybir.AluOpType
ACTF = mybir.ActivationFunctionType

N_CORES = 8
IPC = 4  # images per core
EPS = 1e-12
BF = ml_dtypes.bfloat16


def _build_consts(conv1_w, conv2_w, assign_w, assign_b, lin_w, lin_b):
    c1w = np.asarray(conv1_w, np.float32)
    c2w = np.asarray(conv2_w, np.float32)
    # conv1 banded lhsT, tiles re-anchored to output rows [32r-2, 32r+30):
    # rows p = a*3 + ci (a = h_in - (32r-3), 0..33), cols q = j*4 + co
    # (j = h_out - (32r-2), 0..31). variants: 0 = mid, 1 = first tile
    # (inputs h<0 and outputs h<0 masked), 2 = tail tile r=16 (only
    # outputs 510/511 and inputs 509..511 kept).
    W1 = np.zeros((102, 9, 128), np.float32)
    for dx in range(3):
        for co in range(4):
            for ci in range(3):
                for dy in range(3):
                    for j in range(32):
                        W1[(j + dy) * 3 + ci, dx, j * 4 + co] = c1w[co, ci, dy, dx]
    W1[:, 3:6, :] = W1[:, 0:3, :]
    W1[0:9, 3:6, :] = 0.0        # inputs h=-3..-1
    W1[:, 3:6, 0:8] = 0.0        # outputs h=-2,-1
    W1[:, 6:9, :] = W1[:, 0:3, :]
    W1[9:102, 6:9, :] = 0.0      # inputs h>=512
    W1[:, 6:9, 8:128] = 0.0      # outputs h>=512
    # conv2 banded lhsT with pool1-h fold (input rows are unpooled Y1 rows)
    # and the w-pools stored as SUMs: total scale 0.25. Single variant --
    # image edges are already zeros in the phase-shifted Y1 blocks.
    W2 = np.zeros((80, 3, 128), np.float32)
    for dx in range(3):
        for co in range(16):
            for ci in range(4):
                for dy in range(3):
                    for rr in range(8):
                        for half in range(2):
                            W2[(2 * rr + 2 * dy + half) * 4 + ci, dx, rr * 16 + co] = (
                                0.25 * c2w[co, ci, dy, dx]
                            )
    # logitsT rhs: rows p = rrel*16 + c, cols = q*4 + k (pool2-h fold, V sums)
    aw = np.asarray(assign_w, np.float32)
    A = np.zeros((128, 16), np.float32)
    for q in range(4):
        for k in range(4):
            for c in range(16):
                for half in range(2):
                    A[(2 * q + half) * 16 + c, q * 4 + k] = 0.25 * aw[k, c]
    brep = np.tile(np.asarray(assign_b, np.float32), 4).reshape(16)
    brep128 = np.broadcast_to(brep, (128, 16)).copy()
    return {
        "w1": W1.astype(BF),
        "w2": W2.astype(BF),
        "ablk": A.astype(BF),
        "brep": brep128.astype(np.float32),
        "identb": np.eye(128, dtype=BF),
        "cent": np.zeros(0),  # set by caller (4x centroids)
        "wlin": np.asarray(lin_w, np.float32).T.copy(),  # (64, 7)
        "linb": np.asarray(lin_b, np.float32).reshape(1, 7),
        "ones41": np.ones((4, 1), np.float32),
    }


def _build_program():
    nc = bacc.Bacc("TRN2", target_bir_lowering=False, debug=False,
                   num_devices=N_CORES)
    xin = nc.dram_tensor("x", [IPC, 3, 512, 512], BF16, kind="ExternalInput").ap()
    w1 = nc.dram_tensor("w1", [102, 9, 128], BF16, kind="ExternalInput").ap()
    w2 = nc.dram_tensor("w2", [80, 3, 128], BF16, kind="ExternalInput").ap()
    ablk = nc.dram_tensor("ablk", [128, 16], BF16, kind="ExternalInput").ap()
    brep = nc.dram_tensor("brep", [128, 16], F32, kind="ExternalInput").ap()
    identb = nc.dram_tensor("identb", [128, 128], BF16, kind="ExternalInput").ap()
    cent = nc.dram_tensor("cent", [4, 16], F32, kind="ExternalInput").ap()
    wlin = nc.dram_tensor("wlin", [64, 7], F32, kind="ExternalInput").ap()
    linb = nc.dram_tensor("linb", [1, 7], F32, kind="ExternalInput").ap()
    ones41 = nc.dram_tensor("ones41", [4, 1], F32, kind="ExternalInput").ap()
    out = nc.dram_tensor("out", [IPC, 7], F32, kind="ExternalOutput").ap()

    from contextlib import ExitStack

    with tile.TileContext(nc) as tc, ExitStack() as es:
        consts = es.enter_context(tc.tile_pool(name="consts", bufs=1))
        x1p = es.enter_context(tc.tile_pool(name="x1p", bufs=3))
        y1p = es.enter_context(tc.tile_pool(name="y1p", bufs=2))
        x2p = es.enter_context(tc.tile_pool(name="x2p", bufs=3))
        vp = es.enter_context(tc.tile_pool(name="vp", bufs=2))
        xftp = es.enter_context(tc.tile_pool(name="xftp", bufs=2))
        smp = es.enter_context(tc.tile_pool(name="smp", bufs=3))
        finp = es.enter_context(tc.tile_pool(name="finp", bufs=2))
        p1p = es.enter_context(tc.tile_pool(name="p1p", bufs=2, space="PSUM"))
        p2p = es.enter_context(tc.tile_pool(name="p2p", bufs=2, space="PSUM"))
        ltp = es.enter_context(tc.tile_pool(name="ltp", bufs=2, space="PSUM"))
        vtp = es.enter_context(tc.tile_pool(name="vtp", bufs=1, space="PSUM"))
        gramp = es.enter_context(tc.tile_pool(name="gramp", bufs=1, space="PSUM"))

        w1_sb = consts.tile([102, 9, 128], BF16)
        nc.sync.dma_start(out=w1_sb, in_=w1)
        w2_sb = consts.tile([80, 3, 128], BF16)
        nc.sync.dma_start(out=w2_sb, in_=w2)
        ablk_sb = consts.tile([128, 16], BF16)
        nc.sync.dma_start(out=ablk_sb, in_=ablk)
        brep_sb = consts.tile([128, 16], F32)
        nc.sync.dma_start(out=brep_sb, in_=brep)
        identb_sb = consts.tile([128, 128], BF16)
        nc.sync.dma_start(out=identb_sb, in_=identb)
        cent_sb = consts.tile([4, 16], F32)
        nc.sync.dma_start(out=cent_sb, in_=cent)
        wlin_sb = consts.tile([64, 7], F32)
        nc.sync.dma_start(out=wlin_sb, in_=wlin)
        linb_sb = consts.tile([1, 7], F32)
        nc.sync.dma_start(out=linb_sb, in_=linb)
        ones41_sb = consts.tile([4, 1], F32)
        nc.sync.dma_start(out=ones41_sb, in_=ones41)

        for img in range(IPC):
            # ====== conv1: 17 tiles, output rows [32r-2, 32r+30) ==========
            y1 = y1p.tile([128, 17, 258], BF16, tag="y1")
            nc.vector.memset(y1[:, :, 0:1], 0.0)
            nc.vector.memset(y1[:, :, 257:258], 0.0)
            for r in range(17):
                x1 = x1p.tile([102, 514], BF16, tag="x1")
                # edge tiles: rows not covered by the DMA hold stale SBUF
                # bits; the banded weights there are zero but 0*NaN still
                # poisons the accumulation, so clear the whole tile first.
                if r == 0 or r == 16:
                    nc.vector.memset(x1[:, :], 0.0)
                else:
                    nc.vector.memset(x1[:, 0:1], 0.0)
                    nc.vector.memset(x1[:, 513:514], 0.0)
                var1 = 1 if r == 0 else (2 if r == 16 else 0)
                a_lo = 3 if r == 0 else 0
                a_hi = 3 if r == 16 else 34
                base = 32 * r - 3
                x1v = x1.rearrange("(a c) w -> a c w", c=3)
                dmae = nc.sync if r % 2 == 0 else nc.gpsimd
                for ci in range(3):
                    dmae.dma_start(
                        out=x1v[a_lo:a_hi, ci, 1:513],
                        in_=xin[img, ci, base + a_lo : base + a_hi, :],
                    )
                p1 = p1p.tile([128, 512], F32, tag="p1")
                for dx in range(3):
                    nc.tensor.matmul(
                        p1, w1_sb[:, var1 * 3 + dx, :], x1[:, dx : dx + 512],
                        start=(dx == 0), stop=(dx == 2),
                    )
                p1v = p1.rearrange("p (w two) -> p w two", two=2)
                re1 = smp.tile([128, 256], BF16, tag="re1")
                nc.scalar.activation(out=re1, in_=p1v[:, :, 0], func=ACTF.Relu)
                nc.vector.scalar_tensor_tensor(
                    out=y1[:, r, 1:257], in0=p1v[:, :, 1], scalar=0.0, in1=re1,
                    op0=ALU.max, op1=ALU.add,
                )

            # == conv2: even pairs read Y1 blocks directly; odd pairs are
            # == staged with 2 SBUF DMAs each (window straddles two blocks).
            v = vp.tile([128, 32, 128], BF16, tag="v")
            for pi in range(16):
                even = pi < 8
                if even:
                    b = 2 * pi            # tiles 4*pi, 4*pi+2
                    ts = (4 * pi, 4 * pi + 2)
                    rhs = y1[0:80, b : b + 2, :]
                else:
                    oi = pi - 8
                    ts = (4 * oi + 1, 4 * oi + 3)
                    x2 = x2p.tile([80, 2, 258], BF16, tag="x2")
                    for j in range(2):
                        t = ts[j]
                        b = t // 2
                        nc.gpsimd.dma_start(
                            out=x2[0:64, j, :], in_=y1[64:128, b, :])
                        nc.gpsimd.dma_start(
                            out=x2[64:80, j, :], in_=y1[0:16, b + 1, :])
                    rhs = x2[:]
                p2 = p2p.tile([128, 2, 256], F32, tag="p2")
                for dx in range(3):
                    nc.tensor.matmul(
                        p2, w2_sb[:, dx, :], rhs[:, :, dx : dx + 256],
                        start=(dx == 0), stop=(dx == 2),
                    )
                p2v = p2.rearrange("p a (w two) -> p a w two", two=2)
                for j in range(2):
                    t = ts[j]
                    re2 = smp.tile([128, 128], BF16, tag="re2")
                    nc.scalar.activation(
                        out=re2, in_=p2v[:, j, :, 0], func=ACTF.Relu)
                    nc.vector.scalar_tensor_tensor(
                        out=v[:, t, :], in0=p2v[:, j, :, 1], scalar=0.0,
                        in1=re2, op0=ALU.max, op1=ALU.add,
                    )

            # ====== NetVLAD per pair of tiles (g = 0..15 over t=2g,2g+1) ==
            xft = xftp.tile([128, 32, 4, 17], BF16, tag="xft")
            nc.vector.memset(xft[:, :, :, 16:17], 1.0)
            g32 = gramp.tile([128, 68], F32, tag="gfin")
            for g in range(16):
                # the two V tiles, transposed on the PE
                vt2 = vtp.tile([128, 2, 128], BF16, tag="vt2")
                lt2 = ltp.tile([128, 2, 16], F32, tag="lt2")
                for j in range(2):
                    t = 2 * g + j
                    nc.tensor.transpose(vt2[:, j, :], v[:, t, :], identb_sb[:])
                    nc.tensor.matmul(
                        lt2[:, j, :], v[:, t, :], ablk_sb[:],
                        start=True, stop=True,
                    )
                # xf^T: sum adjacent rrel pairs of VT -> [w, q, c] (bf16)
                vtv = vt2.rearrange("w a (q h c) -> w a q c h", q=4, h=2)
                ce = smp.tile([128, 2, 4, 16], BF16, tag="ce")
                nc.vector.tensor_copy(ce, vtv[:, :, :, :, 0])
                nc.vector.scalar_tensor_tensor(
                    out=xft[:, 2 * g : 2 * g + 2, :, 0:16],
                    in0=vtv[:, :, :, :, 1], scalar=0.0, in1=ce,
                    op0=ALU.bypass, op1=ALU.add,
                )
                # softmax over k (free dim), 0.25-scaled into padded bf16 a
                lb = smp.tile([128, 2, 16], F32, tag="lb")
                nc.vector.tensor_add(
                    lb, lt2,
                    brep_sb[:].unsqueeze(1).broadcast_to((128, 2, 16)))
                lbv = lb.rearrange("w a (q k) -> w a q k", k=4)
                mx = smp.tile([128, 2, 4], F32, tag="mx")
                nc.vector.reduce_max(mx, lbv, axis=AX.X)
                ls = smp.tile([128, 2, 4, 4], F32, tag="ls")
                nc.vector.tensor_sub(
                    ls, lbv, mx.unsqueeze(-1).broadcast_to((128, 2, 4, 4)))
                ae = smp.tile([128, 2, 4, 4], F32, tag="ae")
                nc.scalar.activation(out=ae, in_=ls, func=ACTF.Exp)
                zs = smp.tile([128, 2, 4], F32, tag="zs")
                nc.vector.reduce_sum(zs, ae, axis=AX.X)
                rz = smp.tile([128, 2, 4], F32, tag="rz")
                nc.vector.reciprocal(rz, zs)
                apad = smp.tile([128, 2, 4, 32], BF16, tag="apad")
                nc.vector.scalar_tensor_tensor(
                    out=apad[:, :, :, 0:4], in0=ae, scalar=0.25,
                    in1=rz.unsqueeze(-1).broadcast_to((128, 2, 4, 4)),
                    op0=ALU.mult, op1=ALU.mult,
                )
                for j in range(2):
                    t = 2 * g + j
                    nc.tensor.matmul(
                        g32, apad[:, j, :, :].rearrange("p a b -> p (a b)"),
                        xft[:, t, :, :].rearrange("p a b -> p (a b)"),
                        start=(t == 0), stop=(t == 31),
                    )

            # ================= finale =================
            t0_ = finp.tile([4, 17], F32, tag="t0")
            nc.vector.tensor_copy(t0_, g32[0:4, 0:17])
            t1_ = finp.tile([4, 17], F32, tag="t1")
            nc.vector.tensor_add(t1_, t0_, g32[32:36, 17:34])
            t2_ = finp.tile([4, 17], F32, tag="t2")
            nc.vector.tensor_add(t2_, t1_, g32[64:68, 34:51])
            gsb = finp.tile([4, 17], F32, tag="gsb")
            nc.vector.tensor_add(gsb, t2_, g32[96:100, 51:68])
            cb = finp.tile([4, 16], F32, tag="cb")
            nc.vector.tensor_scalar_mul(cb, cent_sb[:], gsb[:, 16:17])
            v4 = finp.tile([4, 16], F32, tag="v4")
            nc.vector.tensor_sub(v4, gsb[:, 0:16], cb)
            sq = finp.tile([4, 16], F32, tag="sq")
            nc.vector.tensor_mul(sq, v4, v4)
            rs = finp.tile([4, 1], F32, tag="rs")
            nc.vector.reduce_sum(rs, sq, axis=AX.X)
            nrm = finp.tile([4, 1], F32, tag="nrm")
            nc.scalar.activation(out=nrm, in_=rs, func=ACTF.Sqrt)
            nrm2 = finp.tile([4, 1], F32, tag="nrm2")
            nc.vector.tensor_scalar_max(nrm2, nrm, EPS)
            rn = finp.tile([4, 1], F32, tag="rn")
            nc.vector.reciprocal(rn, nrm2)
            vn = finp.tile([4, 16], F32, tag="vn")
            nc.vector.tensor_scalar_mul(vn, v4, rn[:])
            sqn = finp.tile([4, 16], F32, tag="sqn")
            nc.vector.tensor_mul(sqn, vn, vn)
            rs2 = finp.tile([4, 1], F32, tag="rs2")
            nc.vector.reduce_sum(rs2, sqn, axis=AX.X)
            tps = gramp.tile([1, 1], F32, tag="gfin")
            nc.tensor.matmul(tps, ones41_sb[:], rs2[:], start=True, stop=True)
            g1 = finp.tile([1, 1], F32, tag="g1")
            nc.scalar.activation(out=g1, in_=tps, func=ACTF.Sqrt)
            g1m = finp.tile([1, 1], F32, tag="g1m")
            nc.vector.tensor_scalar_max(g1m, g1, EPS)
            g2 = finp.tile([1, 1], F32, tag="g2")
            nc.vector.reciprocal(g2, g1m)
            vcol = finp.tile([64, 1], F32, tag="vcol")
            nc.sync.dma_start(out=vcol, in_=vn[:])
            fps = gramp.tile([1, 7], F32, tag="gfin")
            nc.tensor.matmul(fps, vcol[:], wlin_sb[:], start=True, stop=True)
            osb = finp.tile([1, 7], F32, tag="osb")
            nc.vector.scalar_tensor_tensor(
                out=osb, in0=fps, scalar=g2[:], in1=linb_sb[:],
                op0=ALU.mult, op1=ALU.add,
            )
            nc.sync.dma_start(out=out[img : img + 1, :], in_=osb)

    nc.compile()
    return nc


_CACHE = {}


def _make_in_maps(x, conv1_w, conv2_w, centroids, assign_w, assign_b,
                  lin_w, lin_b):
    consts = _build_consts(conv1_w, conv2_w, assign_w, assign_b, lin_w, lin_b)
    # V/xfT are stored as 4x-scaled sums and a is 0.25-scaled, so the
    # centroid term needs abar*4*centroids.
    consts["cent"] = 4.0 * np.asarray(centroids, np.float32)
    xr = np.asarray(x, np.float32).astype(BF)
    in_maps = []
    for c in range(N_CORES):
        m = dict(consts)
        m["x"] = np.ascontiguousarray(xr[c * IPC : (c + 1) * IPC])
        in_maps.append(m)
    return in_maps


def kernel(x, conv1_w, conv1_b, conv2_w, conv2_b, centroids, assign_w,
           assign_b, lin_w, lin_b):
    # conv biases are zero in this problem; the banded matrices fold weights
    # only, so assert the assumption the kernel relies on.
    assert np.abs(np.asarray(conv1_b)).max() == 0.0
    assert np.abs(np.asarray(conv2_b)).max() == 0.0

    if "nc" not in _CACHE:
        _CACHE["nc"] = _build_program()
    nc = _CACHE["nc"]

    in_maps = _make_in_maps(x, conv1_w, conv2_w, centroids, assign_w,
                            assign_b, lin_w, lin_b)
    res = run_bass_kernel_spmd(nc, in_maps, list(range(N_CORES))).results
    return np.concatenate([res[c]["out"] for c in range(N_CORES)], axis=0)


if __name__ == "__main__":
    print("smoke test: building program only")
    _build_program()
    print("ok")


# revision 6
# speedup vs baseline: 1.1652x; 1.1652x over previous
"""Trainium2 Bass kernel for conv-stack + NetVLAD pooling + linear head.

Pure data parallel: 32 images sharded 4-per-core across 8 NeuronCores.

Per-core pipeline (per image), all matmuls bf16 (1 PE pass vs fp32's 3-4):
  conv1 (3->4, 3x3 SAME, 512x512) as banded bf16 matmuls over (row, ci)
  contraction; relu+w-pool-sum fused into one ACT + one DVE op per tile
  -> Y1 [128=(h%32)*4+c, 17 blk, 256 w] (bf16, sum-pooled).
  conv2 (4->16) as banded bf16 matmuls with pool1's h-direction folded into
  the band; tiles paired for 512-wide moving operands; relu + w-pool-sum
  -> V [128=(r%8)*16+c, 32 t, 128 w] (bf16).
  NetVLAD: ONE matmul per tile with moving [ablk | pool-h matrix] computes
  logits^T AND the pool2-h-summed xf^T together -> PSUM [128 w, 80];
  logits drain to SBUF with the bias folded in (DVE), xf^T drains via an
  ACT copy (bf16). Softmax over k=4 batched across all 32 tiles in 6 big
  ops. Gram: 32 chained matmuls with a (stationary, [128,16]) against
  xf^T|ones (moving 68) accumulating [16, 68]; q-diagonal blocks summed on
  DVE; tiny normalize + linear finale. Gram+finale for image i are emitted
  during image i+1 so the PE never waits on the softmax.
"""
import sys

sys.path.insert(0, "/opt/trn_rl_repo")

import numpy as np
import ml_dtypes
import concourse.bacc as bacc
import concourse.tile as tile
from concourse import mybir
from concourse.bass_utils import run_bass_kernel_spmd

F32 = mybir.dt.float32
BF16 = mybir.dt.bfloat16
AX = mybir.AxisListType
ALU = mybir.AluOpType
ACTF = mybir.ActivationFunctionType

N_CORES = 8
IPC = 4  # images per core
EPS = 1e-12
BF = ml_dtypes.bfloat16


def _build_consts(conv1_w, conv2_w, assign_w, assign_b, lin_w, lin_b):
    c1w = np.asarray(conv1_w, np.float32)
    c2w = np.asarray(conv2_w, np.float32)
    # conv1 banded lhsT, tiles re-anchored to output rows [32r-2, 32r+30):
    # rows p = a*3 + ci (a = h_in - (32r-3), 0..33), cols q = j*4 + co
    # (j = h_out - (32r-2), 0..31). variants: 0 = mid, 1 = first tile
    # (inputs h<0 and outputs h<0 masked), 2 = tail tile r=16 (only
    # outputs 510/511 and inputs 509..511 kept).
    W1 = np.zeros((102, 9, 128), np.float32)
    for dx in range(3):
        for co in range(4):
            for ci in range(3):
                for dy in range(3):
                    for j in range(32):
                        W1[(j + dy) * 3 + ci, dx, j * 4 + co] = c1w[co, ci, dy, dx]
    W1[:, 3:6, :] = W1[:, 0:3, :]
    W1[0:9, 3:6, :] = 0.0        # inputs h=-3..-1
    W1[:, 3:6, 0:8] = 0.0        # outputs h=-2,-1
    W1[:, 6:9, :] = W1[:, 0:3, :]
    W1[9:102, 6:9, :] = 0.0      # inputs h>=512
    W1[:, 6:9, 8:128] = 0.0      # outputs h>=512
    # conv2 banded lhsT with pool1-h fold (input rows are unpooled Y1 rows)
    # and the w-pools stored as SUMs: total scale 0.25. Single variant --
    # image edges are already zeros in the phase-shifted Y1 blocks.
    W2 = np.zeros((80, 3, 128), np.float32)
    for dx in range(3):
        for co in range(16):
            for ci in range(4):
                for dy in range(3):
                    for rr in range(8):
                        for half in range(2):
                            W2[(2 * rr + 2 * dy + half) * 4 + ci, dx, rr * 16 + co] = (
                                0.25 * c2w[co, ci, dy, dx]
                            )
    # combined NetVLAD moving operand, rows p = rrel*16 + c:
    #   cols 0:16   logits^T rhs: q*4 + k (pool2-h fold, V sums, 0.25 scale)
    #   cols 16:80  pool2-h pair-sum matrix -> xf^T as 4x-scaled sums
    aw = np.asarray(assign_w, np.float32)
    AB = np.zeros((128, 80), np.float32)
    for q in range(4):
        for k in range(4):
            for c in range(16):
                for half in range(2):
                    AB[(2 * q + half) * 16 + c, q * 4 + k] = 0.25 * aw[k, c]
    for rr in range(8):
        for c in range(16):
            AB[rr * 16 + c, 16 + (rr // 2) * 16 + c] = 1.0
    brep = np.tile(np.asarray(assign_b, np.float32), 4).reshape(16)
    brep128 = np.broadcast_to(brep, (128, 16)).copy()
    return {
        "w1": W1.astype(BF),
        "w2": W2.astype(BF),
        "ablkp": AB.astype(BF),
        "brep": brep128.astype(np.float32),
        "cent": np.zeros(0),  # set by caller (4x centroids)
        "wlin": np.asarray(lin_w, np.float32).T.copy(),  # (64, 7)
        "linb": np.asarray(lin_b, np.float32).reshape(1, 7),
        "ones41": np.ones((4, 1), np.float32),
    }


def _build_program():
    nc = bacc.Bacc("TRN2", target_bir_lowering=False, debug=False,
                   num_devices=N_CORES)
    xin = nc.dram_tensor("x", [IPC, 3, 512, 512], BF16, kind="ExternalInput").ap()
    w1 = nc.dram_tensor("w1", [102, 9, 128], BF16, kind="ExternalInput").ap()
    w2 = nc.dram_tensor("w2", [80, 3, 128], BF16, kind="ExternalInput").ap()
    ablkp = nc.dram_tensor("ablkp", [128, 80], BF16, kind="ExternalInput").ap()
    brep = nc.dram_tensor("brep", [128, 16], F32, kind="ExternalInput").ap()
    cent = nc.dram_tensor("cent", [4, 16], F32, kind="ExternalInput").ap()
    wlin = nc.dram_tensor("wlin", [64, 7], F32, kind="ExternalInput").ap()
    linb = nc.dram_tensor("linb", [1, 7], F32, kind="ExternalInput").ap()
    ones41 = nc.dram_tensor("ones41", [4, 1], F32, kind="ExternalInput").ap()
    out = nc.dram_tensor("out", [IPC, 7], F32, kind="ExternalOutput").ap()

    from contextlib import ExitStack

    with tile.TileContext(nc) as tc, ExitStack() as es:
        consts = es.enter_context(tc.tile_pool(name="consts", bufs=1))
        x1p = es.enter_context(tc.tile_pool(name="x1p", bufs=3))
        y1p = es.enter_context(tc.tile_pool(name="y1p", bufs=2))
        x2p = es.enter_context(tc.tile_pool(name="x2p", bufs=3))
        vp = es.enter_context(tc.tile_pool(name="vp", bufs=2))
        xftp = es.enter_context(tc.tile_pool(name="xftp", bufs=2))
        lp = es.enter_context(tc.tile_pool(name="lp", bufs=2))
        app = es.enter_context(tc.tile_pool(name="app", bufs=2))
        smp = es.enter_context(tc.tile_pool(name="smp", bufs=2))
        finp = es.enter_context(tc.tile_pool(name="finp", bufs=2))
        p1p = es.enter_context(tc.tile_pool(name="p1p", bufs=2, space="PSUM"))
        p2p = es.enter_context(tc.tile_pool(name="p2p", bufs=2, space="PSUM"))
        cmbp = es.enter_context(tc.tile_pool(name="cmbp", bufs=2, space="PSUM"))
        gramp = es.enter_context(tc.tile_pool(name="gramp", bufs=1, space="PSUM"))

        w1_sb = consts.tile([102, 9, 128], BF16)
        nc.sync.dma_start(out=w1_sb, in_=w1)
        w2_sb = consts.tile([80, 3, 128], BF16)
        nc.sync.dma_start(out=w2_sb, in_=w2)
        ablkp_sb = consts.tile([128, 80], BF16)
        nc.sync.dma_start(out=ablkp_sb, in_=ablkp)
        brep_sb = consts.tile([128, 16], F32)
        nc.sync.dma_start(out=brep_sb, in_=brep)
        cent_sb = consts.tile([4, 16], F32)
        nc.sync.dma_start(out=cent_sb, in_=cent)
        wlin_sb = consts.tile([64, 7], F32)
        nc.sync.dma_start(out=wlin_sb, in_=wlin)
        linb_sb = consts.tile([1, 7], F32)
        nc.sync.dma_start(out=linb_sb, in_=linb)
        ones41_sb = consts.tile([4, 1], F32)
        nc.sync.dma_start(out=ones41_sb, in_=ones41)

        def emit_gram_finale(img, xft, apad):
            # gram: apad puts the 4 q-blocks at out partitions {0,32,64,96}
            # (PSUM reads must start at 32-partition boundaries).
            g32 = gramp.tile([128, 68], F32, tag="gfin")
            xfv = xft.rearrange("p t q c -> p t (q c)")
            apv = apad.rearrange("p t q z -> p t (q z)")
            for t in range(32):
                nc.tensor.matmul(
                    g32, apv[:, t, :], xfv[:, t, :],
                    start=(t == 0), stop=(t == 31),
                )
            # sum the 4 q-diagonal blocks -> [4 k, 17 (c|count)]
            t0_ = finp.tile([4, 17], F32, tag="t0")
            nc.vector.tensor_copy(t0_, g32[0:4, 0:17])
            t1_ = finp.tile([4, 17], F32, tag="t1")
            nc.vector.tensor_add(t1_, t0_, g32[32:36, 17:34])
            t2_ = finp.tile([4, 17], F32, tag="t2")
            nc.vector.tensor_add(t2_, t1_, g32[64:68, 34:51])
            gsb = finp.tile([4, 17], F32, tag="gsb")
            nc.vector.tensor_add(gsb, t2_, g32[96:100, 51:68])
            cb = finp.tile([4, 16], F32, tag="cb")
            nc.vector.tensor_scalar_mul(cb, cent_sb[:], gsb[:, 16:17])
            v4 = finp.tile([4, 16], F32, tag="v4")
            nc.vector.tensor_sub(v4, gsb[:, 0:16], cb)
            sq = finp.tile([4, 16], F32, tag="sq")
            nc.vector.tensor_mul(sq, v4, v4)
            rs = finp.tile([4, 1], F32, tag="rs")
            nc.vector.reduce_sum(rs, sq, axis=AX.X)
            nrm = finp.tile([4, 1], F32, tag="nrm")
            nc.scalar.activation(out=nrm, in_=rs, func=ACTF.Sqrt)
            nrm2 = finp.tile([4, 1], F32, tag="nrm2")
            nc.vector.tensor_scalar_max(nrm2, nrm, EPS)
            rn = finp.tile([4, 1], F32, tag="rn")
            nc.vector.reciprocal(rn, nrm2)
            vn = finp.tile([4, 16], F32, tag="vn")
            nc.vector.tensor_scalar_mul(vn, v4, rn[:])
            sqn = finp.tile([4, 16], F32, tag="sqn")
            nc.vector.tensor_mul(sqn, vn, vn)
            rs2 = finp.tile([4, 1], F32, tag="rs2")
            nc.vector.reduce_sum(rs2, sqn, axis=AX.X)
            tps = gramp.tile([1, 1], F32, tag="gfin")
            nc.tensor.matmul(tps, ones41_sb[:], rs2[:], start=True, stop=True)
            g1 = finp.tile([1, 1], F32, tag="g1")
            nc.scalar.activation(out=g1, in_=tps, func=ACTF.Sqrt)
            g1m = finp.tile([1, 1], F32, tag="g1m")
            nc.vector.tensor_scalar_max(g1m, g1, EPS)
            g2 = finp.tile([1, 1], F32, tag="g2")
            nc.vector.reciprocal(g2, g1m)
            vcol = finp.tile([64, 1], F32, tag="vcol")
            nc.sync.dma_start(out=vcol, in_=vn[:])
            fps = gramp.tile([1, 7], F32, tag="gfin")
            nc.tensor.matmul(fps, vcol[:], wlin_sb[:], start=True, stop=True)
            osb = finp.tile([1, 7], F32, tag="osb")
            nc.vector.scalar_tensor_tensor(
                out=osb, in0=fps, scalar=g2[:], in1=linb_sb[:],
                op0=ALU.mult, op1=ALU.add,
            )
            nc.sync.dma_start(out=out[img : img + 1, :], in_=osb)

        pending = None  # (img, xft, a_sb) awaiting gram+finale
        for img in range(IPC):
            if pending is not None:
                emit_gram_finale(*pending)
            # ====== conv1: 17 tiles, output rows [32r-2, 32r+30) ==========
            y1 = y1p.tile([128, 17, 258], BF16, tag="y1")
            nc.vector.memset(y1[:, :, 0:1], 0.0)
            nc.vector.memset(y1[:, :, 257:258], 0.0)
            for r in range(17):
                x1 = x1p.tile([102, 514], BF16, tag="x1")
                # edge tiles: rows not covered by the DMA hold stale SBUF
                # bits; the banded weights there are zero but 0*NaN still
                # poisons the accumulation, so clear the whole tile first.
                if r == 0 or r == 16:
                    nc.vector.memset(x1[:, :], 0.0)
                else:
                    nc.vector.memset(x1[:, 0:1], 0.0)
                    nc.vector.memset(x1[:, 513:514], 0.0)
                var1 = 1 if r == 0 else (2 if r == 16 else 0)
                a_lo = 3 if r == 0 else 0
                a_hi = 3 if r == 16 else 34
                base = 32 * r - 3
                x1v = x1.rearrange("(a c) w -> a c w", c=3)
                dmae = nc.sync if r % 2 == 0 else nc.gpsimd
                for ci in range(3):
                    dmae.dma_start(
                        out=x1v[a_lo:a_hi, ci, 1:513],
                        in_=xin[img, ci, base + a_lo : base + a_hi, :],
                    )
                p1 = p1p.tile([128, 512], F32, tag="p1")
                for dx in range(3):
                    nc.tensor.matmul(
                        p1, w1_sb[:, var1 * 3 + dx, :], x1[:, dx : dx + 512],
                        start=(dx == 0), stop=(dx == 2),
                    )
                p1v = p1.rearrange("p (w two) -> p w two", two=2)
                re1 = smp.tile([128, 256], BF16, tag="re1")
                nc.scalar.activation(out=re1, in_=p1v[:, :, 0], func=ACTF.Relu)
                nc.vector.scalar_tensor_tensor(
                    out=y1[:, r, 1:257], in0=p1v[:, :, 1], scalar=0.0, in1=re1,
                    op0=ALU.max, op1=ALU.add,
                )

            # == conv2: even pairs read Y1 blocks directly; odd pairs are
            # == staged with 2 SBUF DMAs each (window straddles two blocks).
            v = vp.tile([128, 32, 128], BF16, tag="v")
            vv = v.rearrange("p (u two) w -> p u two w", two=2)
            for pi in range(16):
                even = pi < 8
                if even:
                    b = 2 * pi            # tiles 4*pi, 4*pi+2
                    oi, par = pi, 0
                    rhs = y1[0:80, b : b + 2, :]
                else:
                    oi, par = pi - 8, 1
                    ts = (4 * oi + 1, 4 * oi + 3)
                    x2 = x2p.tile([80, 2, 258], BF16, tag="x2")
                    for j in range(2):
                        b = ts[j] // 2
                        nc.gpsimd.dma_start(
                            out=x2[0:64, j, :], in_=y1[64:128, b, :])
                        nc.gpsimd.dma_start(
                            out=x2[64:80, j, :], in_=y1[0:16, b + 1, :])
                    rhs = x2[:]
                p2 = p2p.tile([128, 2, 256], F32, tag="p2")
                for dx in range(3):
                    nc.tensor.matmul(
                        p2, w2_sb[:, dx, :], rhs[:, :, dx : dx + 256],
                        start=(dx == 0), stop=(dx == 2),
                    )
                # relu + w-pool-sum for both tiles of the pair in one
                # ACT + one DVE op (out tiles 2*oi, 2*oi+1 at parity par)
                p2v = p2.rearrange("p a (w two) -> p a w two", two=2)
                re2 = smp.tile([128, 2, 128], BF16, tag="re2")
                nc.scalar.activation(
                    out=re2, in_=p2v[:, :, :, 0], func=ACTF.Relu)
                nc.vector.scalar_tensor_tensor(
                    out=vv[:, 2 * oi : 2 * oi + 2, par, :],
                    in0=p2v[:, :, :, 1], scalar=0.0,
                    in1=re2, op0=ALU.max, op1=ALU.add,
                )

            # ====== NetVLAD combined logits+xf^T matmuls, 8 groups of 4 ==
            xft = xftp.tile([128, 32, 4, 17], BF16, tag="xft")
            nc.vector.memset(xft[:, :, :, 16:17], 1.0)
            L = lp.tile([128, 32, 16], F32, tag="L")
            for gi in range(8):
                pc = cmbp.tile([128, 4, 128], F32, tag="pc")
                for j in range(4):
                    t = 4 * gi + j
                    nc.tensor.matmul(
                        pc[:, j, 0:80], v[:, t, :], ablkp_sb[:],
                        start=True, stop=True,
                    )
                nc.vector.scalar_tensor_tensor(
                    out=L[:, 4 * gi : 4 * gi + 4, :], in0=pc[:, :, 0:16],
                    scalar=0.0,
                    in1=brep_sb[:].unsqueeze(1).broadcast_to((128, 4, 16)),
                    op0=ALU.bypass, op1=ALU.add,
                )
                pcx = pc[:, :, 16:80].rearrange("p a (q c) -> p a q c", c=16)
                nc.scalar.activation(
                    out=xft[:, 4 * gi : 4 * gi + 4, :, 0:16], in_=pcx,
                    func=ACTF.Copy,
                )

            # ====== softmax over k, batched across all 32 tiles ==========
            Lv = L.rearrange("p t (q k) -> p (t q) k", k=4)
            mx = smp.tile([128, 128], F32, tag="mx")
            nc.vector.reduce_max(mx, Lv, axis=AX.X)
            ls = smp.tile([128, 128, 4], F32, tag="ls")
            nc.vector.tensor_sub(
                ls, Lv, mx.unsqueeze(-1).broadcast_to((128, 128, 4)))
            ae = smp.tile([128, 128, 4], F32, tag="ae")
            nc.scalar.activation(out=ae, in_=ls, func=ACTF.Exp)
            zs = smp.tile([128, 128], F32, tag="zs")
            nc.vector.reduce_sum(zs, ae, axis=AX.X)
            rz = smp.tile([128, 128], F32, tag="rz")
            nc.vector.reciprocal(rz, zs)
            apad = app.tile([128, 32, 4, 32], BF16, tag="a")
            if img < 2:
                # zero the padding once per pool buffer; later images reuse
                # the same two buffers and only rewrite cols 0:4.
                nc.vector.memset(apad[:, :, :, :], 0.0)
            av = apad.rearrange("p t q z -> p (t q) z")
            nc.vector.scalar_tensor_tensor(
                out=av[:, :, 0:4], in0=ae, scalar=0.25,
                in1=rz.unsqueeze(-1).broadcast_to((128, 128, 4)),
                op0=ALU.mult, op1=ALU.mult,
            )
            pending = (img, xft, apad)

        emit_gram_finale(*pending)

    nc.compile()
    return nc


_CACHE = {}


def _make_in_maps(x, conv1_w, conv2_w, centroids, assign_w, assign_b,
                  lin_w, lin_b):
    consts = _build_consts(conv1_w, conv2_w, assign_w, assign_b, lin_w, lin_b)
    # V/xfT are stored as 4x-scaled sums and a is 0.25-scaled, so the
    # centroid term needs abar*4*centroids.
    consts["cent"] = 4.0 * np.asarray(centroids, np.float32)
    xr = np.asarray(x, np.float32).astype(BF)
    in_maps = []
    for c in range(N_CORES):
        m = dict(consts)
        m["x"] = np.ascontiguousarray(xr[c * IPC : (c + 1) * IPC])
        in_maps.append(m)
    return in_maps


def kernel(x, conv1_w, conv1_b, conv2_w, conv2_b, centroids, assign_w,
           assign_b, lin_w, lin_b):
    # conv biases are zero in this problem; the banded matrices fold weights
    # only, so assert the assumption the kernel relies on.
    assert np.abs(np.asarray(conv1_b)).max() == 0.0
    assert np.abs(np.asarray(conv2_b)).max() == 0.0

    if "nc" not in _CACHE:
        _CACHE["nc"] = _build_program()
    nc = _CACHE["nc"]

    in_maps = _make_in_maps(x, conv1_w, conv2_w, centroids, assign_w,
                            assign_b, lin_w, lin_b)
    res = run_bass_kernel_spmd(nc, in_maps, list(range(N_CORES))).results
    return np.concatenate([res[c]["out"] for c in range(N_CORES)], axis=0)


if __name__ == "__main__":
    print("smoke test: building program only")
    _build_program()
    print("ok")
